# revision 22
# baseline (speedup 1.0000x reference)
"""Trainium2 Bass kernel for an attention seq2seq decoder (nn_Decoder).

Reference math (per batch row b):
  att_h = eout @ wW.T + wb
  scan over L-1 steps t:
    x = [emb[y_t], ctx]; h,c = LSTM(x, h, c; att_Wih, att_Whh, att_b)
    state = h @ vW.T + vb
    scores = sum(w_att_v * tanh(state + att_h), -1) + mbias
    alpha = softmax(scores); ctx = alpha @ eout
  att_fea = [h_t*ym, ctx_t*ym]
  dec scan: dh_t = LSTM(att_fea_t; dec_*)
  logit = ([att_fea, dh] * ym) @ cls_W.T + cls_b

Distribution: data-parallel over batch B=64 across 8 cores (8 rows/core),
all parameters replicated; the timestep scans stay local per core.

Device layout notes (per core, b = 8 local rows split in 2 groups of 4):
 - everything elementwise lives on partitions 0..3 (per group) or 0..7
   (merged dec LSTM) so DVE/ACT ops stay partition-aligned.
 - eout resident in SBUF as [128(t%128), b, t//128, d] bf16 (ctx matmuls)
 - att_h resident transposed [128(d%128), dchunk, b, t] bf16 so the per-step
   tanh(state + att_h) fuses the add into the ACT bias (state is [d,1] cols).
 - sigmoids are computed as 0.5*(1+tanh(z/2)) by pre-halving i/f/o weight
   rows on the host, so the whole kernel needs one ACT table set (tanh+exp).
 - the cell state is stored as cH = c/2 and hidden as hH = 2h with the 0.5
   factors folded into Whh/vW on the host.
 - precomputed per-step gate rows (embedding part + bias) are injected into
   the PSUM accumulation via "selector" matmuls (identity-slice stationary).
"""

import numpy as np
import ml_dtypes
from dataclasses import dataclass

import concourse.bass as bass
import concourse.bacc as bacc
import concourse.tile as tile
import concourse.mybir as mybir
from concourse.masks import make_identity

F32 = mybir.dt.float32
BF16 = mybir.dt.bfloat16
AF = mybir.ActivationFunctionType
OP = mybir.AluOpType
BF = ml_dtypes.bfloat16

D = 256  # model dim (layout hardcodes D == 2*128)


@dataclass(frozen=True)
class Cfg:
    T: int = 1024          # encoder length
    L: int = 65            # decoder length (steps = L-1)
    V: int = 4235          # vocab
    BL: int = 8            # batch rows per core
    num_devices: int = 8
    with_mbias: bool = False

    @property
    def NS(self):
        return self.L - 1

    @property
    def NT(self):
        return self.NS * self.BL  # total (t,g,b) rows

    @property
    def TC(self):
        return self.T // 128

    @property
    def G(self):
        return 2

    @property
    def GB(self):
        return self.BL // 2


def build_program(cfg: Cfg):
    NS, NT, T, V, TC = cfg.NS, cfg.NT, cfg.T, cfg.V, cfg.TC
    BL, G, GB = cfg.BL, cfg.G, cfg.GB
    assert BL == 8 and GB == 4
    assert T % 128 == 0 and NT % 64 == 0 and NS % 8 == 0
    NTC = (NT + 127) // 128       # row chunks of pregates
    MC = NT // 128                # classifier row chunks (NT multiple of 64)
    NB = NS // 8                  # dec batches of 8 steps
    TH = max(1, T // 512)         # score halves
    THN = min(T, 512)             # elements per score chunk
    NV = (V + 511) // 512

    nc = bacc.Bacc("TRN2", target_bir_lowering=False, debug=False,
                   num_devices=cfg.num_devices)

    def din(name, shape, dt=BF16):
        return nc.dram_tensor(name, shape, dt, kind="ExternalInput").ap()

    eout_d = din("eout_r", [128, BL, TC, D])
    embr_d = din("embr", [128, NTC, D])
    wihe_d = din("wihe", [128, 2, 1024])
    wihc_d = din("wihc", [128, 2, 1024])
    whh_d = din("whh", [128, 2, 1024])
    attb_d = din("attb", [1, 1024])
    vw_d = din("vw", [128, 2, 2, 128])
    ww_d = din("ww", [128, 2, 2, 128])
    biasvw_d = din("biasvw", [128, 2], F32)
    wattv_d = din("wattv", [128, 2, 4, 4])
    dwih_d = din("dwih", [128, 4, 1024])
    dwhh_d = din("dwhh", [128, 2, 1024])
    decb_d = din("decb", [1, 1024])
    cls_d = din("cls", [128, 6, V])
    clsb_d = din("clsb", [1, V])
    ymh_d = din("ymh", [4, G, NS], F32)   # 0.5*ym, partitions 0..3
    ymf_d = din("ymf", [4, G, NS], F32)   # ym
    ymh8_d = din("ymh8", [8, NS], F32)    # 0.5*ym rows (g*4+bb)
    if cfg.with_mbias:
        mbias_d = din("mbias", [4, G, T], F32)
    out_d = nc.dram_tensor("logits", [MC, 128, V], F32,
                           kind="ExternalOutput").ap()

    with tile.TileContext(nc) as tc:
        import contextlib
        stack = contextlib.ExitStack()
        with stack:
            singles = stack.enter_context(tc.tile_pool(name="singles", bufs=1))

            # ---------- persistent SBUF ----------
            eout_sb = singles.tile([128, BL, TC, D], BF16)
            atth_sb = singles.tile([128, 2, BL, T], BF16)
            pregates_sb = singles.tile([128, NTC, 1024], BF16)
            decpre_sb = singles.tile([128, NTC, 1024], BF16)
            affT_sb = singles.tile([128, 4, NT], BF16)
            dhT_sb = singles.tile([128, 2, NT], BF16)
            clsb_sb = singles.tile([1, V], BF16)
            wihe_sb = singles.tile([128, 2, 1024], BF16)
            wihc_sb = singles.tile([128, 2, 1024], BF16)
            whh_sb = singles.tile([128, 2, 1024], BF16)
            attb_sb = singles.tile([1, 1024], BF16)
            vw_sb = singles.tile([128, 2, 2, 128], BF16)
            ww_sb = singles.tile([128, 2, 2, 128], BF16)
            biasvw_sb = singles.tile([128, 2], F32)
            wattv_sb = singles.tile([128, 2, 4, 4], BF16)
            alT4A_sb = singles.tile([128, TC, 4, 4], BF16)
            alT4B_sb = singles.tile([128, TC, 4, 4], BF16)
            dwih_sb = singles.tile([128, 4, 1024], BF16)
            dwhh_sb = singles.tile([128, 2, 1024], BF16)
            decb_sb = singles.tile([1, 1024], BF16)
            embr_sb = singles.tile([128, NTC, D], BF16)
            embT_sb = singles.tile([128, 2, NT], BF16)
            ymh_sb = singles.tile([4, G, NS], F32)
            ymf_sb = singles.tile([4, G, NS], F32)
            ymh8_sb = singles.tile([8, NS], F32)
            ident = singles.tile([128, 128], BF16)
            ones_sb = singles.tile([1, 128], BF16)
            if cfg.with_mbias:
                mbias_sb = singles.tile([4, G, T], F32)

            # recurrent state
            hHT_sb = singles.tile([128, 2, BL], BF16)    # 2h, transposed
            ctxT_sb = singles.tile([128, 2, BL], BF16)   # ctx, transposed
            state_sb = singles.tile([128, 2, BL], F32)   # vW@h + vb + wb
            cA_sb = singles.tile([4, D], F32)            # c/2 per group
            cB_sb = singles.tile([4, D], F32)
            hdT_sb = singles.tile([128, 2, 8], BF16)     # dec 2h transposed
            cdec_sb = singles.tile([8, D], F32)          # dec c/2

            # ---------- input DMAs ----------
            for dst, src in [
                (eout_sb, eout_d), (embr_sb, embr_d), (wihe_sb, wihe_d),
                (wihc_sb, wihc_d), (whh_sb, whh_d), (attb_sb, attb_d),
                (vw_sb, vw_d), (ww_sb, ww_d), (biasvw_sb, biasvw_d),
                (wattv_sb, wattv_d), (dwih_sb, dwih_d), (dwhh_sb, dwhh_d),
                (decb_sb, decb_d), (clsb_sb, clsb_d),
                (ymh_sb, ymh_d), (ymf_sb, ymf_d), (ymh8_sb, ymh8_d),
            ]:
                nc.sync.dma_start(out=dst[:], in_=src)
            if cfg.with_mbias:
                nc.sync.dma_start(out=mbias_sb[:], in_=mbias_d)

            make_identity(nc, ident[:])
            nc.vector.memset(ones_sb[:], 1.0)
            nc.vector.memset(alT4A_sb[:], 0.0)
            nc.vector.memset(alT4B_sb[:], 0.0)
            nc.vector.memset(decpre_sb[:], 0.0)
            nc.vector.memset(hHT_sb[:], 0.0)
            nc.vector.memset(ctxT_sb[:], 0.0)
            nc.vector.memset(hdT_sb[:], 0.0)
            nc.vector.memset(cA_sb[:], 0.0)
            nc.vector.memset(cB_sb[:], 0.0)
            nc.vector.memset(cdec_sb[:], 0.0)

            # ---------- prep phase ----------
            with tc.tile_pool(name="prep_ps", bufs=3, space="PSUM") as pps, \
                 tc.tile_pool(name="prep_ps2", bufs=3, space="PSUM") as pps2, \
                 tc.tile_pool(name="prep_sb", bufs=3) as psb:
                # embT: transpose embr row-chunks -> [d, row]
                for m in range(NTC):
                    mrows = min(128, NT - m * 128)
                    for ch in range(2):
                        tp = pps.tile([128, 128], BF16, tag="tp")
                        nc.tensor.transpose(
                            tp[:, 0:mrows],
                            embr_sb[0:mrows, m, ch * 128:(ch + 1) * 128],
                            ident[0:mrows, 0:mrows])
                        nc.vector.tensor_copy(
                            embT_sb[:, ch, m * 128:m * 128 + mrows], tp[:, 0:mrows])
                # att pregates = embed @ WihE.T + att_b  -> [row, 1024]
                for m in range(NTC):
                    mrows = min(128, NT - m * 128)
                    for half in range(2):
                        gp = pps2.tile([128, 512], F32, tag="gp")
                        nc.tensor.matmul(
                            gp[0:mrows, :], ones_sb[0:1, 0:mrows],
                            attb_sb[0:1, half * 512:(half + 1) * 512],
                            start=True, stop=False)
                        for kc in range(2):
                            nc.tensor.matmul(
                                gp[0:mrows, :],
                                embT_sb[:, kc, m * 128:m * 128 + mrows],
                                wihe_sb[:, kc, half * 512:(half + 1) * 512],
                                start=False, stop=(kc == 1))
                        nc.vector.tensor_copy(
                            pregates_sb[0:mrows, m, half * 512:(half + 1) * 512],
                            gp[0:mrows, :])
                # att_h (transposed): per b, eoutT then wW @ eoutT
                for b in range(BL):
                    eoutT_b = psb.tile([128, 2, T], BF16, tag="eoutT")
                    for t_c in range(TC):
                        for ch in range(2):
                            tp2 = pps.tile([128, 128], BF16, tag="tp")
                            nc.tensor.transpose(
                                tp2[:],
                                eout_sb[:, b, t_c, ch * 128:(ch + 1) * 128],
                                ident[:])
                            nc.vector.tensor_copy(
                                eoutT_b[:, ch, t_c * 128:(t_c + 1) * 128],
                                tp2[:])
                    for mc2 in range(2):
                        for n in range(T // 512 if T >= 512 else 1):
                            nn = min(512, T)
                            ap = pps2.tile([128, 512], F32, tag="gp")
                            for kc in range(2):
                                nc.tensor.matmul(
                                    ap[:, 0:nn],
                                    ww_sb[:, kc, mc2, :],
                                    eoutT_b[:, kc, n * 512:n * 512 + nn],
                                    start=(kc == 0), stop=(kc == 1))
                            nc.vector.tensor_copy(
                                atth_sb[:, mc2, b, n * 512:n * 512 + nn],
                                ap[:, 0:nn])

            # ---------- scan phase ----------
            with tc.tile_pool(name="ps1", bufs=2, space="PSUM") as ps1, \
                 tc.tile_pool(name="ps_big", bufs=2, space="PSUM") as psbig, \
                 tc.tile_pool(name="psT", bufs=2, space="PSUM") as psT, \
                 tc.tile_pool(name="sc_sb", bufs=2) as scsb, \
                 tc.tile_pool(name="tanh_sb", bufs=3) as tsb:

                cgrp = [cA_sb, cB_sb]

                def lstm(g, t):
                    """gates -> h,c update -> hHT, state for group g step t."""
                    g4 = g * 4
                    r0 = t * 8 + g * 4
                    gates = psbig.tile([4, 1024], F32, tag="gates")
                    for half in range(2):
                        hs = slice(half * 512, (half + 1) * 512)
                        nc.tensor.matmul(gates[:, hs],
                                         ident[:, r0 % 128:r0 % 128 + 4],
                                         pregates_sb[:, r0 // 128, hs],
                                         start=True, stop=False)
                        for kc in range(2):
                            nc.tensor.matmul(gates[:, hs],
                                             ctxT_sb[:, kc, g4:g4 + 4],
                                             wihc_sb[:, kc, hs],
                                             start=False, stop=False)
                        for kc in range(2):
                            nc.tensor.matmul(gates[:, hs],
                                             hHT_sb[:, kc, g4:g4 + 4],
                                             whh_sb[:, kc, hs],
                                             start=False, stop=(kc == 1))
                    tg = scsb.tile([4, 1024], BF16, tag="tg")
                    nc.scalar.activation(tg[:], gates[:], AF.Tanh)
                    # c' = 0.5*(1+tf)*c + 0.25*(1+ti)*tg   (c stored as c/2)
                    c_sb = cgrp[g]
                    ti = tg[:, 0:256]
                    tf = tg[:, 256:512]
                    tgg = tg[:, 512:768]
                    to = tg[:, 768:1024]
                    aT = scsb.tile([4, D], F32, tag="aT")
                    bT = scsb.tile([4, D], F32, tag="bT")
                    nc.vector.scalar_tensor_tensor(aT[:], tf, 1.0, c_sb[:],
                                                   OP.add, OP.mult)
                    nc.vector.scalar_tensor_tensor(bT[:], ti, 1.0, tgg,
                                                   OP.add, OP.mult)
                    nc.vector.scalar_tensor_tensor(aT[:], bT[:], 0.5, aT[:],
                                                   OP.mult, OP.add)
                    nc.vector.tensor_scalar_mul(c_sb[:], aT[:], 0.5)
                    tc_bf = scsb.tile([4, D], BF16, tag="tcb")
                    nc.scalar.activation(tc_bf[:], c_sb[:], AF.Tanh, scale=2.0)
                    hH = scsb.tile([4, D], BF16, tag="hH")
                    nc.vector.scalar_tensor_tensor(hH[:], to, 1.0, tc_bf[:],
                                                   OP.add, OP.mult)
                    # transposes: hHT
                    hTp = psT.tile([128, 2, 4], BF16, tag="psT")
                    for ch in range(2):
                        nc.tensor.transpose(hTp[:, ch, :],
                                            hH[:, ch * 128:(ch + 1) * 128],
                                            ident[0:4, 0:4])
                    nc.vector.tensor_copy(hHT_sb[:, :, g4:g4 + 4], hTp[:])
                    # att_fea h-part (h*ym), transposed into affT
                    afh = scsb.tile([4, D], BF16, tag="afh")
                    nc.vector.tensor_scalar(afh[:], hH[:],
                                            ymh_sb[:, g, t:t + 1], None, OP.mult)
                    afp = psT.tile([128, 2, 4], BF16, tag="psT")
                    for ch in range(2):
                        nc.tensor.transpose(afp[:, ch, :],
                                            afh[:, ch * 128:(ch + 1) * 128],
                                            ident[0:4, 0:4])
                    nc.vector.tensor_copy(affT_sb[:, 0:2, r0:r0 + 4], afp[:])
                    # state = vW @ hHT + (vb + wb)
                    stp = ps1.tile([128, 2, 4], F32, tag="ps1")
                    for mc2 in range(2):
                        for kc in range(2):
                            nc.tensor.matmul(stp[:, mc2, :],
                                             vw_sb[:, kc, mc2, :],
                                             hHT_sb[:, kc, g4:g4 + 4],
                                             start=(kc == 0), stop=(kc == 1))
                    for mc2 in range(2):
                        nc.vector.tensor_scalar(state_sb[:, mc2, g4:g4 + 4],
                                                stp[:, mc2, :],
                                                biasvw_sb[:, mc2:mc2 + 1],
                                                None, OP.add)

                def tanh_reduce_exp(g, t):
                    g4 = g * 4
                    th_t0 = tsb.tile([128, 4, T], BF16, tag="th")
                    th_t1 = tsb.tile([128, 4, T], BF16, tag="th")
                    th_ts = (th_t0, th_t1)
                    for bb in range(4):
                        for dc in range(2):
                            nc.scalar.activation(
                                th_ts[dc][:, bb, 0:T], atth_sb[:, dc, g4 + bb, :],
                                AF.Tanh, bias=state_sb[:, dc, g4 + bb:g4 + bb + 1])
                    exp_t = scsb.tile([4, T], BF16, tag="exp")
                    esum = scsb.tile([4, TH], F32, tag="esum")
                    for th in range(TH):
                        sc = ps1.tile([4, 512], F32, tag="ps1")
                        for bb in range(4):
                            for dc in range(2):
                                nc.tensor.matmul(
                                    sc[:, 0:THN],
                                    wattv_sb[:, dc, bb, :],
                                    th_ts[dc][:, bb, th * 512:th * 512 + THN],
                                    start=(bb == 0 and dc == 0),
                                    stop=(bb == 3 and dc == 1))
                        if cfg.with_mbias:
                            scs = scsb.tile([4, 512], F32, tag="scs")
                            nc.vector.tensor_tensor(
                                scs[:, 0:THN], sc[:, 0:THN],
                                mbias_sb[:, g, th * 512:th * 512 + THN], OP.add)
                            src = scs[:, 0:THN]
                        else:
                            src = sc[:, 0:THN]
                        nc.scalar.activation(
                            exp_t[:, th * 512:th * 512 + THN], src, AF.Exp,
                            accum_out=esum[:, th:th + 1])
                    rcp = scsb.tile([4, 1], F32, tag="rcp")
                    if TH > 1:
                        ssum = scsb.tile([4, 1], F32, tag="ssum")
                        nc.vector.tensor_tensor(ssum[:], esum[:, 0:1],
                                                esum[:, 1:2], OP.add)
                        nc.vector.reciprocal(rcp[:], ssum[:])
                    else:
                        nc.vector.reciprocal(rcp[:], esum[:, 0:1])
                    return exp_t, rcp

                def finish_attention(g, t, exp_t, rcp):
                    """alpha transposes, ctx matmuls, ctx scaling + stores."""
                    g4 = g * 4
                    r0 = t * 8 + g * 4
                    alp = psT.tile([128, TC, 4], BF16, tag="psT")
                    for t_c in range(TC):
                        nc.tensor.transpose(alp[:, t_c, :],
                                            exp_t[:, t_c * 128:(t_c + 1) * 128],
                                            ident[0:4, 0:4])
                    alT4 = alT4A_sb if g == 0 else alT4B_sb
                    diag = bass.AP(tensor=alT4.tensor, offset=alT4.offset,
                                   ap=[alT4.ap[0], [16, TC], [5, 4]])
                    nc.vector.tensor_copy(diag, alp[:])
                    cxp = ps1.tile([4, D], F32, tag="ps1")
                    for t_c in range(TC):
                        for bb in range(4):
                            nc.tensor.matmul(cxp[:],
                                             alT4[:, t_c, bb, :],
                                             eout_sb[:, g4 + bb, t_c, :],
                                             start=(t_c == 0 and bb == 0),
                                             stop=(t_c == TC - 1 and bb == 3))
                    ctx_bf = scsb.tile([4, D], BF16, tag="ctx_bf")
                    nc.vector.tensor_scalar(ctx_bf[:], cxp[:], rcp[:], None,
                                            OP.mult)
                    cTp = psT.tile([128, 2, 4], BF16, tag="psT")
                    for ch in range(2):
                        nc.tensor.transpose(cTp[:, ch, :],
                                            ctx_bf[:, ch * 128:(ch + 1) * 128],
                                            ident[0:4, 0:4])
                    nc.vector.tensor_copy(ctxT_sb[:, :, g4:g4 + 4], cTp[:])
                    # att_fea ctx part (ctx*ym) -> affT cols
                    afc = scsb.tile([4, D], BF16, tag="afc")
                    nc.vector.tensor_scalar(afc[:], ctx_bf[:],
                                            ymf_sb[:, g, t:t + 1], None, OP.mult)
                    afp2 = psT.tile([128, 2, 4], BF16, tag="psT")
                    for ch in range(2):
                        nc.tensor.transpose(afp2[:, ch, :],
                                            afc[:, ch * 128:(ch + 1) * 128],
                                            ident[0:4, 0:4])
                    nc.vector.tensor_copy(affT_sb[:, 2:4, r0:r0 + 4], afp2[:])

                def dec_pregates(k):
                    """batched dec input projection for steps 8k..8k+7."""
                    c0 = 64 * k
                    p0 = c0 % 128
                    m = c0 // 128
                    for half in range(2):
                        hs = slice(half * 512, (half + 1) * 512)
                        dp = psbig.tile([128, 512], F32, tag="gates")
                        nc.tensor.matmul(dp[p0:p0 + 64, :],
                                         ones_sb[0:1, 0:64],
                                         decb_sb[0:1, hs],
                                         start=True, stop=False)
                        for ch in range(4):
                            nc.tensor.matmul(dp[p0:p0 + 64, :],
                                             affT_sb[:, ch, c0:c0 + 64],
                                             dwih_sb[:, ch, hs],
                                             start=False, stop=(ch == 3))
                        nc.vector.tensor_copy(decpre_sb[p0:p0 + 64, m, hs],
                                              dp[p0:p0 + 64, :])

                def dec_step(u):
                    r0 = u * 8
                    gates = psbig.tile([8, 1024], F32, tag="gates")
                    for half in range(2):
                        hs = slice(half * 512, (half + 1) * 512)
                        nc.tensor.matmul(gates[:, hs],
                                         ident[:, r0 % 128:r0 % 128 + 8],
                                         decpre_sb[:, r0 // 128, hs],
                                         start=True, stop=False)
                        for kc in range(2):
                            nc.tensor.matmul(gates[:, hs],
                                             hdT_sb[:, kc, :],
                                             dwhh_sb[:, kc, hs],
                                             start=False, stop=(kc == 1))
                    tg = scsb.tile([8, 1024], BF16, tag="tg")
                    nc.scalar.activation(tg[:], gates[:], AF.Tanh)
                    ti = tg[:, 0:256]
                    tf = tg[:, 256:512]
                    tgg = tg[:, 512:768]
                    to = tg[:, 768:1024]
                    aT = scsb.tile([8, D], F32, tag="aT")
                    bT = scsb.tile([8, D], F32, tag="bT")
                    nc.vector.scalar_tensor_tensor(aT[:], tf, 1.0, cdec_sb[:],
                                                   OP.add, OP.mult)
                    nc.vector.scalar_tensor_tensor(bT[:], ti, 1.0, tgg,
                                                   OP.add, OP.mult)
                    nc.vector.scalar_tensor_tensor(aT[:], bT[:], 0.5, aT[:],
                                                   OP.mult, OP.add)
                    nc.vector.tensor_scalar_mul(cdec_sb[:], aT[:], 0.5)
                    tc_bf = scsb.tile([8, D], BF16, tag="tcb")
                    nc.scalar.activation(tc_bf[:], cdec_sb[:], AF.Tanh,
                                         scale=2.0)
                    hH = scsb.tile([8, D], BF16, tag="hH")
                    nc.vector.scalar_tensor_tensor(hH[:], to, 1.0, tc_bf[:],
                                                   OP.add, OP.mult)
                    hTp = psT.tile([128, 2, 8], BF16, tag="psT")
                    for ch in range(2):
                        nc.tensor.transpose(hTp[:, ch, :],
                                            hH[:, ch * 128:(ch + 1) * 128],
                                            ident[0:8, 0:8])
                    nc.vector.tensor_copy(hdT_sb[:], hTp[:])
                    # dh store: (h*ym).T -> dhT cols
                    dhm = scsb.tile([8, D], BF16, tag="dhm")
                    nc.vector.tensor_scalar(dhm[:], hH[:],
                                            ymh8_sb[:, u:u + 1], None, OP.mult)
                    dTp = psT.tile([128, 2, 8], BF16, tag="psT")
                    for ch in range(2):
                        nc.tensor.transpose(dTp[:, ch, :],
                                            dhm[:, ch * 128:(ch + 1) * 128],
                                            ident[0:8, 0:8])
                    nc.vector.tensor_copy(dhT_sb[:, :, r0:r0 + 8], dTp[:])

                # software-pipelined main loop: B runs half a step behind A
                pend = {}  # g -> (t, exp_t, rcp)
                dec_done = 0

                def half(g, t):
                    nonlocal dec_done
                    lstm(g, t)
                    og = 1 - g
                    if og in pend:
                        pt, e, r = pend.pop(og)
                        finish_attention(og, pt, e, r)
                    exp_t, rcp = tanh_reduce_exp(g, t)
                    pend[g] = (t, exp_t, rcp)

                for t in range(NS):
                    half(0, t)
                    half(1, t)
                    if (t + 1) % 8 == 0:
                        # dec batch k needs all of att_fea rows <= t: flush
                        # group B's pending attention first.
                        pt, e, r = pend.pop(1)
                        finish_attention(1, pt, e, r)
                        dec_pregates((t + 1) // 8 - 1)
                    # one dec step per att step, lagging 8 behind
                    if t >= 8:
                        dec_step(dec_done)
                        dec_done += 1
                for g2 in list(pend):
                    pt, e, r = pend.pop(g2)
                    finish_attention(g2, pt, e, r)
                for u in range(dec_done, NS):
                    dec_step(u)

            # ---------- classifier (cls_W streamed from DRAM) ----------
            with tc.tile_pool(name="cls_ps", bufs=4, space="PSUM") as cps, \
                 tc.tile_pool(name="cls_w", bufs=2) as cwp, \
                 tc.tile_pool(name="cls_sb2", bufs=4) as csb:
                for nv in range(NV):
                    nn = min(512, V - nv * 512)
                    ns = slice(nv * 512, nv * 512 + nn)
                    wt = cwp.tile([128, 6, 512], BF16, tag="wt")
                    for ch in range(6):
                        nc.sync.dma_start(out=wt[:, ch, 0:nn],
                                          in_=cls_d[:, ch, ns])
                    for m in range(MC):
                        ms = slice(m * 128, (m + 1) * 128)
                        lp = cps.tile([128, 512], F32, tag="lp")
                        nc.tensor.matmul(lp[:, 0:nn], ones_sb[0:1, :],
                                         clsb_sb[0:1, ns],
                                         start=True, stop=False)
                        for ch in range(4):
                            nc.tensor.matmul(lp[:, 0:nn], affT_sb[:, ch, ms],
                                             wt[:, ch, 0:nn],
                                             start=False, stop=False)
                        for ch in range(2):
                            nc.tensor.matmul(lp[:, 0:nn], dhT_sb[:, ch, ms],
                                             wt[:, 4 + ch, 0:nn],
                                             start=False, stop=(ch == 1))
                        lsb = csb.tile([128, 512], F32, tag="lsb")
                        if m % 2 == 0:
                            nc.vector.tensor_copy(lsb[:, 0:nn], lp[:, 0:nn])
                        else:
                            nc.scalar.copy(lsb[:, 0:nn], lp[:, 0:nn])
                        nc.sync.dma_start(out=out_d[m, :, ns],
                                          in_=lsb[:, 0:nn])

    nc.compile()
    return nc


# ---------------------------------------------------------------------------
# host marshaling
# ---------------------------------------------------------------------------

def host_prep_shared(cfg: Cfg, emb, att_Wih, att_Whh, att_b, wW, wb, vW, vb,
                     w_att_v, dec_Wih, dec_Whh, dec_b, cls_W, cls_b):
    """Weight preprocessing shared by all cores."""
    f = np.float32
    att_Wih = np.asarray(att_Wih, f).copy()
    att_Whh = np.asarray(att_Whh, f).copy()
    att_b = np.asarray(att_b, f).copy()
    dec_Wih = np.asarray(dec_Wih, f).copy()
    dec_Whh = np.asarray(dec_Whh, f).copy()
    dec_b = np.asarray(dec_b, f).copy()
    # sigmoid(z) = 0.5*(1+tanh(z/2)): halve i,f,o rows (gate order i,f,g,o)
    ifo = np.r_[0:512, 768:1024]
    for W in (att_Wih, dec_Wih, att_Whh, dec_Whh):
        W[ifo] *= 0.5
    for bvec in (att_b, dec_b):
        bvec[ifo] *= 0.5
    # hidden state stored as 2h: halve all h-consuming weights
    att_Whh *= 0.5
    dec_Whh *= 0.5
    vW05 = np.asarray(vW, f) * 0.5

    def pack_kn(WT, kc):  # [K, N] -> [128, kc, N]
        K, N = WT.shape
        assert K == kc * 128
        return np.ascontiguousarray(
            WT.reshape(kc, 128, N).transpose(1, 0, 2)).astype(BF)

    wihe = pack_kn(att_Wih[:, 0:256].T, 2)
    wihc = pack_kn(att_Wih[:, 256:512].T, 2)
    whh = pack_kn(att_Whh.T, 2)
    dwih = pack_kn(dec_Wih.T, 4)
    dwhh = pack_kn(dec_Whh.T, 2)

    def pack_kmn(WT):  # [256, 256] -> [128, kc2, mc2, 128]
        return np.ascontiguousarray(
            WT.reshape(2, 128, 2, 128).transpose(1, 0, 2, 3)).astype(BF)

    vw = pack_kmn(vW05.T)
    ww = pack_kmn(np.asarray(wW, f).T)
    biasvw = np.ascontiguousarray(
        (np.asarray(vb, f) + np.asarray(wb, f)).reshape(2, 128).T)
    wv = np.asarray(w_att_v, f).reshape(2, 128).T      # [128, dc]
    wattv = np.zeros((128, 2, 4, 4), f)
    for bb in range(4):
        wattv[:, :, bb, bb] = wv
    wattv = wattv.astype(BF)
    cls = np.ascontiguousarray(
        np.asarray(cls_W, f).T.reshape(6, 128, cfg.V).transpose(1, 0, 2)
    ).astype(BF)
    shared = dict(
        wihe=wihe, wihc=wihc, whh=whh,
        attb=att_b.reshape(1, 1024).astype(BF),
        vw=vw, ww=ww, biasvw=biasvw.astype(f), wattv=wattv,
        dwih=dwih, dwhh=dwhh, decb=dec_b.reshape(1, 1024).astype(BF),
        cls=cls, clsb=np.asarray(cls_b, f).reshape(1, cfg.V).astype(BF),
    )
    return shared


def host_prep_core(cfg: Cfg, c, eout, x_mask, y, y_mask, emb, shared):
    """Per-core input shards. b rows c*BL .. c*BL+BL."""
    f = np.float32
    BL, T, NS, TC, NT = cfg.BL, cfg.T, cfg.NS, cfg.TC, cfg.NT
    NTC = (NT + 127) // 128
    sl = slice(c * BL, (c + 1) * BL)
    e = np.asarray(eout[sl], f)                       # [BL, T, D]
    eout_r = np.ascontiguousarray(
        e.reshape(BL, TC, 128, D).transpose(2, 0, 1, 3)).astype(BF)
    yv = np.asarray(y[sl])                            # [BL, L]
    embed = np.asarray(emb, f)[yv[:, :-1]]            # [BL, NS, D]
    # rows r = t*8 + g*4 + bb  (b_local = g*4+bb)
    embed_r = np.ascontiguousarray(
        embed.transpose(1, 0, 2).reshape(NT, D))      # [(t,b), D]
    embr = np.ascontiguousarray(
        embed_r.reshape(NTC, 128, D).transpose(1, 0, 2)).astype(BF)
    ym = np.asarray(y_mask[sl], f)[:, 1:]             # [BL, NS]
    ymh8 = np.ascontiguousarray(0.5 * ym)
    ymh = np.ascontiguousarray((0.5 * ym).reshape(2, 4, NS).transpose(1, 0, 2))
    ymf = np.ascontiguousarray(ym.reshape(2, 4, NS).transpose(1, 0, 2))
    d = dict(shared)
    d.update(eout_r=eout_r, embr=embr, ymh8=ymh8.astype(f),
             ymh=ymh.astype(f), ymf=ymf.astype(f))
    if cfg.with_mbias:
        mb = (np.asarray(x_mask[sl], f)[..., 0] - 1.0) * 1e30  # [BL, T]
        d["mbias"] = np.ascontiguousarray(
            mb.reshape(2, 4, T).transpose(1, 0, 2)).astype(f)
    return d


def host_post(cfg: Cfg, outs):
    """Reassemble [MC,128,V] per-core row-major (t,b) results -> [B, NS, V]."""
    parts = []
    for o in outs:
        lg = o.reshape(cfg.NT, cfg.V).reshape(cfg.NS, cfg.BL, cfg.V)
        parts.append(np.ascontiguousarray(lg.transpose(1, 0, 2)))
    return np.concatenate(parts, axis=0)


_PROG_CACHE = {}


def _get_program(cfg: Cfg):
    if cfg not in _PROG_CACHE:
        _PROG_CACHE[cfg] = build_program(cfg)
    return _PROG_CACHE[cfg]


def run(cfg: Cfg, inputs, trace=False):
    from concourse.bass_utils import run_bass_kernel_spmd
    nc = _get_program(cfg)
    shared = host_prep_shared(
        cfg, inputs["emb"], inputs["att_Wih"], inputs["att_Whh"],
        inputs["att_b"], inputs["wW"], inputs["wb"], inputs["vW"],
        inputs["vb"], inputs["w_att_v"], inputs["dec_Wih"],
        inputs["dec_Whh"], inputs["dec_b"], inputs["cls_W"], inputs["cls_b"])
    in_maps = [
        host_prep_core(cfg, c, inputs["eout"], inputs["x_mask"], inputs["y"],
                       inputs["y_mask"], inputs["emb"], shared)
        for c in range(cfg.num_devices)
    ]
    res = run_bass_kernel_spmd(nc, in_maps,
                               core_ids=list(range(cfg.num_devices)),
                               trace=trace)
    out = host_post(cfg, [res.results[c]["logits"]
                          for c in range(cfg.num_devices)])
    return out, res


def kernel(**inputs):
    x_mask = np.asarray(inputs["x_mask"], np.float32)
    cfg = Cfg(with_mbias=not bool((x_mask == 1.0).all()))
    out, _ = run(cfg, inputs)
    return out
